# revision 42
# baseline (speedup 1.0000x reference)
"""Multi-head causal attention on 8 Trainium2 NeuronCores.

Sharding: core c handles batch b = c//2 and head-group g = c%2 (4 heads).
Each core computes a partial output  y_part = sum_{h in group} attn_h @ wo_h
for its batch; the host sums the two head-group partials per batch and adds bo.
Pure data-SPMD: one NEFF, per-core input slices, no collectives.

Per-core kernel layout choices:
  - Inputs are host-transposed to x^T [D=512, S=2048] so the projection
    contraction dim (D) lands on SBUF partitions with contiguous DMA.
  - Q/K projections are head-pair-packed: out psum [128 = 2x64 dims, q].
  - Scores are computed transposed (S^T[k, q] blocks) so the exp output P^T
    is directly the moving operand of the P@V matmul - no on-chip transposes.
  - P^T is stored fp8e4m3 (exp writes it directly); the P@V matmul runs in
    DoubleRow perf mode: lhsT = (V_hi, V_lo) fp8 pair (hi/lo split of bf16 V,
    computed on GPSIMD), rhs = P^T duplicated via a stride-0 AP. One DR
    matmul per (j, head) does the full-precision-V P@V at half PE cost.
    A 16th fp8 column of 1.0 in V_hi emits the softmax denominator row.
  - Causal: block-skipping at 128-row granularity, q-range trimming per
    diagonal block. The diagonal 128x128 is masked by a uint32 bitwise-AND
    on DVE (zeroes fp8 bytes above the diagonal). The attn accumulation
    group grows in width as j decreases (has_written overwrite semantics),
    so no memsets are needed for the trimmed regions.
  - y is written bf16, one batched DMA per 512-row slab.
"""

import os
import sys

import ml_dtypes
import numpy as np

try:
    import concourse.bass as bass
except ImportError:  # pragma: no cover
    sys.path.insert(0, "/opt/trn_rl_repo")
    import concourse.bass as bass

import concourse.bacc as bacc
import concourse.mybir as mybir
import concourse.tile as tile
from concourse.bass_utils import run_bass_kernel_spmd

F32 = mybir.dt.float32
F32R = mybir.dt.float32r
BF16 = mybir.dt.bfloat16
F8 = mybir.dt.float8e4
U32 = mybir.dt.uint32

S = 2048          # sequence length
D = 512           # model dim
DK = 64           # head dim (= dv)
NH = 4            # heads per core
NPAIR = 2         # head pairs per core
P = 128           # partitions
NQB = S // 512    # 512-wide q column blocks
NKB = S // P      # 128-wide k row blocks
NSB = S // P      # 128-wide s row blocks
NCHUNK = D // P   # 128-row chunks of the model dim
VW = 80           # padded fp8 V' row width (65 used; stride % 16 == 0)


def build_nc(has_mask: bool, cfg: dict | None = None) -> bass.Bass:
    cfg = {**{"sc_bufs": 2, "at_bufs": 1, "pt_bufs": 14, "nrm_bufs": 4, "shift_dma": "sync", "prefetch": 8}, **(cfg or {})}
    nc = bacc.Bacc("TRN2", target_bir_lowering=False)

    qT = nc.dram_tensor("qt", [D, S], BF16, kind="ExternalInput")
    kT = nc.dram_tensor("kt", [D, S], BF16, kind="ExternalInput")
    vT = nc.dram_tensor("vt", [D, S], BF16, kind="ExternalInput")
    wq = nc.dram_tensor("wq", [D, NH * DK], BF16, kind="ExternalInput")
    wk = nc.dram_tensor("wk", [D, NH * DK], BF16, kind="ExternalInput")
    wv = nc.dram_tensor("wv", [D, NH * DK], BF16, kind="ExternalInput")
    bq = nc.dram_tensor("bq", [P, NPAIR], F32, kind="ExternalInput")
    bk = nc.dram_tensor("bk", [P, NPAIR], F32, kind="ExternalInput")
    bv = nc.dram_tensor("bv", [NH * DK], F32, kind="ExternalInput")
    wo = nc.dram_tensor("wo", [NPAIR, P, D], BF16, kind="ExternalInput")
    tri = nc.dram_tensor("tri", [P, P // 4], U32, kind="ExternalInput")
    onef = nc.dram_tensor("onef", [DK], F32R, kind="ExternalInput")
    ebias = nc.dram_tensor("ebias", [1], F32, kind="ExternalInput")
    y = nc.dram_tensor("y", [S, D], BF16, kind="ExternalOutput")

    y4 = y.rearrange("(i b p) d -> i p b d", p=P, b=4)

    with tile.TileContext(nc) as tc, nc.allow_low_precision(reason="fp8 P/V attention path; bf16 outputs"):
        with tc.tile_pool(name="persist", bufs=1) as persist:
            QT = persist.tile([P, NPAIR, S], F8, tag="QT")
            KT = persist.tile([P, NPAIR, S], BF16, tag="KT")
            # fp8 K hi/lo pairs: [dims, pair, j, {hi,lo}, 128]
            KT8 = persist.tile([P, NPAIR, NKB, 2, P], F8, tag="KT8")
            Vp = persist.tile([P, NKB, NH, DK], F32, tag="Vp")
            # fp8 V' hi/lo pairs: [k, j, {hi,lo}, head, 80] (cols 0:64 = V, col 64 = 1/0)
            Vp8 = persist.tile([P, NKB, 2, NH, VW], F8, tag="Vp8")
            aT0 = persist.tile([P, S], BF16, tag="aT0")
            aT1 = persist.tile([P, S], BF16, tag="aT1")
            attnT = [aT0, aT1]
            wo_sb = persist.tile([P, NPAIR, D], BF16, tag="wo")
            bq_sb = persist.tile([P, NPAIR], F32, tag="bq")
            bk_sb = persist.tile([P, NPAIR], F32, tag="bk")
            bv_sb = persist.tile([P, NH, DK], F32, tag="bv")
            tri_sb = persist.tile([P, P // 4], U32, tag="tri")
            onef_sb = persist.tile([DK + 1, DK], F32R, tag="onef")
            eb_sb = persist.tile([P, 1], F32, tag="eb")

            with (
                tc.tile_pool(name="inp", bufs=1) as inp,
                tc.tile_pool(name="wts", bufs=1) as wts,
                tc.tile_pool(name="scps", bufs=cfg["sc_bufs"], space="PSUM") as scps,
                tc.tile_pool(name="atps", bufs=cfg["at_bufs"], space="PSUM") as atps,
                tc.tile_pool(name="prepps", bufs=2, space="PSUM") as prepps,
                tc.tile_pool(name="ptp", bufs=cfg["pt_bufs"]) as ptp,
                tc.tile_pool(name="nrm", bufs=cfg["nrm_bufs"]) as nrm,
                tc.tile_pool(name="outp", bufs=2) as outp,
            ):
                qT_sb = inp.tile([P, NCHUNK, S], BF16, tag="qT")
                kT_sb = inp.tile([P, NCHUNK, S], BF16, tag="kT")
                vT_sb = inp.tile([P, NCHUNK, S], BF16, tag="vT")
                wq_sb = wts.tile([P, NCHUNK, NH * DK], BF16, tag="wq")
                wk_sb = wts.tile([P, NCHUNK, NH * DK], BF16, tag="wk")
                wv_sb = wts.tile([P, NCHUNK, NH * DK], BF16, tag="wv")
                # One combined DMA per (tensor, 512-col block), emitted in
                # criticality order: HWDGE charges a serial per-DMA trigger
                # cost, so few big transfers with the slab-0 data first.
                kT4 = kT.rearrange("(c p) s -> p c s", p=P)
                qT4 = qT.rearrange("(c p) s -> p c s", p=P)
                vT4 = vT.rearrange("(c p) s -> p c s", p=P)

                def in_dma(dst, src, qb):
                    cols = slice(512 * qb, 512 * (qb + 1))
                    nc.sync.dma_start(out=dst[..., cols], in_=src[..., cols])

                # warm the PE pstate ramp during the input-DMA wait: ~4us of
                # garbage matmuls on a memset tile into a scratch psum bank
                wu = ptp.tile([P, 512], BF16, tag="wu", name="wu")
                nc.gpsimd.memset(wu, 0.0)
                wups = prepps.tile([P, 512], F32, tag="pp", name="wups")
                for _ in range(12):
                    nc.tensor.matmul(wups, lhsT=wu[:, 0:P], rhs=wu, start=True, stop=True)
                # preload the Exp activation table off the critical path
                wua = ptp.tile([1, 1], BF16, tag="wua", name="wua")
                with tc.high_priority(offset=(1 << 20) + 1000):
                    nc.scalar.activation(out=wua, in_=wu[0:1, 0:1], func=mybir.ActivationFunctionType.Exp)

                nc.sync.dma_start(out=wk_sb, in_=wk.rearrange("(c p) m -> p c m", p=P))
                in_dma(kT_sb, kT4, 0)
                nc.sync.dma_start(out=wq_sb, in_=wq.rearrange("(c p) m -> p c m", p=P))
                in_dma(qT_sb, qT4, 0)
                nc.sync.dma_start(out=bq_sb, in_=bq[:, :])
                nc.sync.dma_start(out=bk_sb, in_=bk[:, :])
                nc.sync.dma_start(out=wv_sb, in_=wv.rearrange("(c p) m -> p c m", p=P))
                in_dma(vT_sb, vT4, 0)
                nc.sync.dma_start(out=bv_sb, in_=bv.rearrange("(h c) -> h c", h=NH).partition_broadcast(P))
                nc.sync.dma_start(out=tri_sb, in_=tri[:, :])
                nc.sync.dma_start(out=onef_sb[DK : DK + 1, :], in_=onef[:].partition_broadcast(1))
                nc.sync.dma_start(out=eb_sb, in_=ebias[:].partition_broadcast(P))
                # V' ones/zeros columns via cheap GPSIMD memsets (a strided
                # 1-byte DMA here would serialize the startup DMA queue)
                nc.gpsimd.memset(Vp8[:, :, 0, :, DK], 1.0)
                nc.gpsimd.memset(Vp8[:, :, 1, :, DK], 0.0)
                for qb in (1, 2):
                    in_dma(kT_sb, kT4, qb)
                    in_dma(qT_sb, qT4, qb)
                    in_dma(vT_sb, vT4, qb)
                nc.sync.dma_start(out=wo_sb, in_=wo.rearrange("e p m -> p e m"))
                for qb in (3,):
                    in_dma(kT_sb, kT4, qb)
                    in_dma(qT_sb, qT4, qb)
                    in_dma(vT_sb, vT4, qb)

                # pair-packed projection of one 512-col block: psum [128, 512]
                def proj_block(x_sb, w_sb, b_sb, out_sb, pair, qb):
                    ps = prepps.tile([P, 512], F32, tag="pp", name="pj")
                    for c in range(NCHUNK):
                        nc.tensor.matmul(
                            ps,
                            lhsT=(w_sb[:, c, P * pair : P * (pair + 1)]),
                            rhs=(x_sb[:, c, 512 * qb : 512 * (qb + 1)]),
                            start=(c == 0),
                            stop=(c == NCHUNK - 1),
                        )
                    # the bias-evac feeds the scores pipeline: rank it with
                    # the pipeline so DVE doesn't sit on it behind other work
                    with tc.high_priority(offset=1 << 20):
                        nc.vector.tensor_scalar_add(
                            out=out_sb[:, pair, 512 * qb : 512 * (qb + 1)],
                            in0=ps,
                            scalar1=b_sb[:, pair : pair + 1],
                        )

                # fp8 hi/lo split of K block (pair, qb) on GPSIMD; also a
                # scores-pipeline producer, so pipeline priority
                def ksplit_block(pair, qb):
                    kb = KT[:, pair, 512 * qb : 512 * (qb + 1)].rearrange(
                        "p (j c) -> p j c", c=P
                    )
                    with tc.high_priority(offset=1 << 20):
                        nc.gpsimd.tensor_copy(
                            out=KT8[:, pair, 4 * qb : 4 * qb + 4, 0, :], in_=kb
                        )
                        nc.gpsimd.tensor_tensor(
                            out=KT8[:, pair, 4 * qb : 4 * qb + 4, 1, :],
                            in0=kb,
                            in1=KT8[:, pair, 4 * qb : 4 * qb + 4, 0, :],
                            op=mybir.AluOpType.subtract,
                        )

                # V natural: per s-block psum [128 s, 256 = 4 heads x 64]
                def vproj_block(sb):
                    ps_full = prepps.tile([P, 512], F32, tag="pp", name="pjv")
                    ps = ps_full[:, 0 : NH * DK]
                    for c in range(NCHUNK):
                        nc.tensor.matmul(
                            ps,
                            lhsT=(vT_sb[:, c, P * sb : P * (sb + 1)]),
                            rhs=(wv_sb[:, c]),
                            start=(c == 0),
                            stop=(c == NCHUNK - 1),
                        )
                    nc.vector.tensor_tensor(
                        out=Vp[:, sb],
                        in0=ps.rearrange("p (h c) -> p h c", h=NH),
                        in1=bv_sb,
                        op=mybir.AluOpType.add,
                    )

                # fp8 hi/lo split of V' block sb (GPSIMD, SBUF-only)
                def vsplit_block(sb):
                    nc.gpsimd.tensor_copy(
                        out=Vp8[:, sb, 0, :, 0:DK], in_=Vp[:, sb]
                    )
                    nc.gpsimd.tensor_tensor(
                        out=Vp8[:, sb, 1, :, 0:DK],
                        in0=Vp[:, sb],
                        in1=Vp8[:, sb, 0, :, 0:DK],
                        op=mybir.AluOpType.subtract,
                    )

                # output projection for one 128-row s-block into the slab tile
                def oproj_block(sb, y_sb):
                    yp = prepps.tile([P, 512], F32, tag="pp", name="yp")
                    for pair in range(NPAIR):
                        nc.tensor.matmul(
                            yp,
                            lhsT=(attnT[pair][:, P * sb : P * (sb + 1)]),
                            rhs=(wo_sb[:, pair]),
                            start=(pair == 0),
                            stop=(pair == NPAIR - 1),
                        )
                    nc.vector.tensor_copy(out=y_sb[:, sb % 4, :], in_=yp)

                # ---- attention slabs with dripped prep work ----
                # work items are (due, weight, fn): fn must be EMITTED before
                # slab `due` starts (Tile derives deps from emission order).
                # The pacer spreads items evenly over the global j-slot
                # budget, weighted by each item's PE cost, so per-j PE load
                # stays just under the ACT exp cadence.
                work = []
                total_jslots = 2 * sum(4 * i + 4 for i in range(NQB)) if has_mask else 2 * NQB * NKB
                pace = {"jdone": 0, "popped_w": 0.0, "queued_w": 0.0}

                def drip_due(I):
                    while work and work[0][0] <= I:
                        pace["popped_w"] += work[0][1]
                        work.pop(0)[2]()

                def drip_pace():
                    pace["jdone"] += 1
                    target = pace["queued_w"] * pace["jdone"] / total_jslots
                    while work and pace["popped_w"] < target:
                        pace["popped_w"] += work[0][1]
                        work.pop(0)[2]()

                def queue(due, weight, fn):
                    work.append((due, weight, fn))
                    pace["queued_w"] += weight

                def scores_exp(pair, I, j):
                    t = j - 4 * I
                    qlo = 128 * t if (has_mask and t > 0) else 0
                    sc = scps.tile([P, 2, 512], F32, tag="sc", name="sc")
                    pt = ptp.tile([P, 2, 512], F8, tag="pt", name="pt")
                    for hh in range(2):
                        qv = QT[64 * hh : 64 * hh + 64, pair, 512 * I + qlo : 512 * (I + 1)]
                        nc.tensor.matmul(
                            sc[:, hh, qlo:],
                            lhsT=KT8[64 * hh : 64 * hh + 64, pair, j, :, :],
                            rhs=qv.unsqueeze(1).broadcast_to([64, 2, 512 - qlo]),
                            start=True,
                            stop=True,
                            tile_position=(64 * hh, 0),
                            perf_mode=mybir.MatmulPerfMode.DoubleRow,
                        )
                    # one ACT op covers both heads; scores are x32 (fp8 Q was
                    # prescaled), compensated by the exp scale; the +ln32 bias
                    # recenters P in the fp8 range and cancels in the softmax
                    nc.scalar.activation(
                        out=pt[:, :, qlo:],
                        in_=sc[:, :, qlo:],
                        func=mybir.ActivationFunctionType.Exp,
                        bias=eb_sb[:, 0:1],
                        scale=0.03125,
                    )
                    if has_mask and 0 <= t:
                        for hh in range(2):
                            nc.vector.tensor_tensor(
                                out=pt[:, hh, qlo : qlo + P].bitcast(U32),
                                in0=pt[:, hh, qlo : qlo + P].bitcast(U32),
                                in1=tri_sb,
                                op=mybir.AluOpType.bitwise_and,
                            )
                    return (pt, qlo)

                def attn_mm(at, pair, I, j, jmax, ptq):
                    pt, qlo = ptq
                    for hh in range(2):
                        rhs = pt[:, hh, qlo:].unsqueeze(1).broadcast_to([P, 2, 512 - qlo])
                        nc.tensor.matmul(
                            at[0 : DK + 1, hh, qlo:],
                            lhsT=Vp8[:, j, :, 2 * pair + hh, 0 : DK + 1],
                            rhs=rhs,
                            start=(j == jmax),
                            stop=(j == 0),
                            perf_mode=mybir.MatmulPerfMode.DoubleRow,
                        )

                def normalize(at, pair, I):
                    # reciprocal straight off the psum denominator row while
                    # the numerators evacuate; the broadcast ones-matmul then
                    # REUSES the at psum banks (already drained), so the
                    # scores-pool rotation never blocks on this chain
                    rec = nrm.tile([DK + 1, 2, 512], F32R, tag="rec")
                    nc.vector.reciprocal(out=rec[DK : DK + 1], in_=at[DK : DK + 1])
                    anum = nrm.tile([DK, 2, 512], F32, tag="anum")
                    if pair == 1 and I == NQB - 1:
                        # tail: ACT is idle; shortens the last at-release chain
                        nc.scalar.copy(out=anum, in_=at[0:DK])
                    else:
                        nc.vector.tensor_copy(out=anum, in_=at[0:DK])
                    for hh in range(2):
                        nc.tensor.matmul(
                            at[0:DK, hh, :],
                            lhsT=onef_sb[DK : DK + 1, :],
                            rhs=rec[DK : DK + 1, hh, :],
                            start=True,
                            stop=True,
                            tile_position=(DK, 0),
                        )
                    nc.vector.tensor_tensor(
                        out=attnT[pair][0:DK, 512 * I : 512 * (I + 1)],
                        in0=anum[0:DK, 0],
                        in1=at[0:DK, 0],
                        op=mybir.AluOpType.mult,
                    )
                    tmp = nrm.tile([DK, 512], BF16, tag="tmp")
                    if I == NQB - 1:
                        # tail: per-chunk multiply+shift so the final output
                        # projections start as soon as each chunk lands
                        for cc in range(4):
                            cs = slice(128 * cc, 128 * (cc + 1))
                            nc.vector.tensor_tensor(
                                out=tmp[:, cs],
                                in0=anum[0:DK, 1, cs],
                                in1=at[0:DK, 1, cs],
                                op=mybir.AluOpType.mult,
                            )
                            getattr(nc, cfg["shift_dma"]).dma_start(
                                out=attnT[pair][DK:P, 512 * I + 128 * cc : 512 * I + 128 * (cc + 1)],
                                in_=tmp[:, cs],
                            )
                    else:
                        nc.vector.tensor_tensor(
                            out=tmp,
                            in0=anum[0:DK, 1],
                            in1=at[0:DK, 1],
                            op=mybir.AluOpType.mult,
                        )
                        getattr(nc, cfg["shift_dma"]).dma_start(
                            out=attnT[pair][DK:P, 512 * I : 512 * (I + 1)], in_=tmp
                        )

                carry = {}

                def slab(pair, I):
                    jmax = 4 * I + 3 if has_mask else NKB - 1
                    js = list(range(jmax, -1, -1))

                    def get_pt(j):
                        key = (pair, I, j)
                        if key in carry:
                            return carry.pop(key)
                        return scores_exp(pair, I, j)

                    at = atps.tile([P, 2, 512], F32, tag="at", name="at")
                    for j in js:
                        with tc.high_priority(offset=1 << 20):
                            pt = get_pt(j)
                            attn_mm(at, pair, I, j, jmax, pt)
                    with tc.high_priority(offset=1 << 20):
                        normalize(at, pair, I)

                # prologue: pair-0 K/Q for slab 0 first (exp critical
                # path), then V prep, then ALL remaining projections in slab
                # order. Everything here is low priority: the scheduler
                # backfills PE with it whenever the high-priority attention
                # pipeline has nothing ready.
                k_qbs0 = [0] if has_mask else list(range(NQB))
                v_sbs0 = list(range(4)) if has_mask else list(range(NSB))
                def ksplit_chunked(pair, qb):
                    for j in range(4 * qb + 3, 4 * qb - 1, -1):
                        kb = KT[:, pair, P * j : P * (j + 1)]
                        with tc.high_priority(offset=1 << 20):
                            nc.gpsimd.tensor_copy(out=KT8[:, pair, j, 0, :], in_=kb)
                            nc.gpsimd.tensor_tensor(
                                out=KT8[:, pair, j, 1, :],
                                in0=kb,
                                in1=KT8[:, pair, j, 0, :],
                                op=mybir.AluOpType.subtract,
                            )

                for qb in k_qbs0:
                    proj_block(kT_sb, wk_sb, bk_sb, KT, 0, qb)
                    ksplit_chunked(0, qb)
                proj_block(qT_sb, wq_sb, bq_sb, QT, 0, 0)
                for sb in reversed(v_sbs0):
                    # descending: the slab-0 attn loop consumes j=jmax first
                    vproj_block(sb)
                    vsplit_block(sb)
                for qb in k_qbs0:
                    proj_block(kT_sb, wk_sb, bk_sb, KT, 1, qb)
                    ksplit_block(1, qb)
                proj_block(qT_sb, wq_sb, bq_sb, QT, 1, 0)
                for I in range(1, NQB):
                    if has_mask:
                        proj_block(kT_sb, wk_sb, bk_sb, KT, 0, I)
                        ksplit_block(0, I)
                        proj_block(qT_sb, wq_sb, bq_sb, QT, 0, I)
                        proj_block(kT_sb, wk_sb, bk_sb, KT, 1, I)
                        ksplit_block(1, I)
                        for sb in range(4 * I, 4 * I + 4):
                            vproj_block(sb)
                            vsplit_block(sb)
                        proj_block(qT_sb, wq_sb, bq_sb, QT, 1, I)
                    else:
                        proj_block(qT_sb, wq_sb, bq_sb, QT, 0, I)
                        proj_block(qT_sb, wq_sb, bq_sb, QT, 1, I)

                for I in range(NQB):
                    # oproj/DMA for slab I-1 were queued after its normalize
                    # was emitted (Tile derives deps from emission order)
                    drip_due(I)
                    slab(0, I)
                    slab(1, I)
                    if I > 0:
                        ys = outp.tile([P, 4, 512], BF16, tag="y", name="ys")
                        for sb in range(4 * (I - 1), 4 * I):
                            queue(I + 1, 430, lambda sb=sb, ys=ys: oproj_block(sb, ys))
                        queue(I + 1, 0, lambda I=I, ys=ys: nc.sync.dma_start(out=y4[I - 1], in_=ys))
                    if I + 1 < NQB:
                        # prefetch the next slab's first scores/exp blocks so
                        # ACT keeps streaming across the slab boundary
                        jm = 4 * (I + 1) + 3 if has_mask else NKB - 1
                        npf = cfg.get("prefetch", 8)
                        for j in range(jm, max(jm - npf, -1), -1):
                            with tc.high_priority(offset=1 << 20):
                                carry[(0, I + 1, j)] = scores_exp(0, I + 1, j)
                # drain any remaining prep, then the final slab's output
                # proj: ACT evacuation (idle at the tail) + per-block DMAs.
                # Junk matmuls keep the PE pstate warm through the last
                # normalize chain so the final oprojs run at full rate.
                for _ in range(10):
                    nc.tensor.matmul(wups, lhsT=wu[:, 0:P], rhs=wu, start=True, stop=True)
                drip_due(NQB + 1)
                ys = outp.tile([P, 4, 512], BF16, tag="y", name="ys")
                for sb in range(4 * (NQB - 1), 4 * NQB):
                    yp = prepps.tile([P, 512], F32, tag="pp", name="yp")
                    for pair in range(NPAIR):
                        nc.tensor.matmul(
                            yp,
                            lhsT=(attnT[pair][:, P * sb : P * (sb + 1)]),
                            rhs=(wo_sb[:, pair]),
                            start=(pair == 0),
                            stop=(pair == NPAIR - 1),
                        )
                    nc.scalar.copy(out=ys[:, sb % 4, :], in_=yp)
                    nc.sync.dma_start(out=y4[NQB - 1][:, sb % 4], in_=ys[:, sb % 4, :])
    nc.compile()
    return nc


_NC_CACHE: dict = {}
LAST_RESULT = None


def _get_nc(has_mask: bool) -> bass.Bass:
    key = bool(has_mask)
    if key not in _NC_CACHE:
        _NC_CACHE[key] = build_nc(key)
    return _NC_CACHE[key]


def _core_inputs(queries, keys, values, wq, bq, wk, bk, wv, bv, wo, core: int) -> dict:
    b, g = core // 2, core % 2
    heads = list(range(4 * g, 4 * g + 4))
    f = np.float32
    bf = ml_dtypes.bfloat16
    f8 = ml_dtypes.float8_e4m3
    scale = np.float32(1.0 / np.sqrt(DK))

    def packw(w, s=1.0):
        return np.ascontiguousarray(
            np.concatenate([w[h] for h in heads], axis=1) * np.float32(s)
        ).astype(bf)

    def packb(bvec, s=1.0):
        cols = [np.concatenate([bvec[heads[2 * p]], bvec[heads[2 * p + 1]]]) for p in range(NPAIR)]
        return np.ascontiguousarray(np.stack(cols, axis=1) * np.float32(s), dtype=f)

    return {
        "qt": np.ascontiguousarray(queries[b].T).astype(bf),
        "kt": np.ascontiguousarray(keys[b].T).astype(bf),
        "vt": np.ascontiguousarray(values[b].T).astype(bf),
        "wq": packw(wq, scale * 32.0),
        "wk": packw(wk),
        "wv": packw(wv),
        "bq": packb(bq, scale * 32.0),
        "bk": packb(bk),
        "bv": np.ascontiguousarray(np.concatenate([bv[h] for h in heads]), dtype=f),
        "wo": np.ascontiguousarray(
            np.stack([wo[DK * heads[2 * p] : DK * (heads[2 * p] + 2)] for p in range(NPAIR)])
        ).astype(bf),
        "tri": np.ascontiguousarray(
            np.triu(np.full((P, P), 0xFF, dtype=np.uint8))
        ).view(np.uint32),
        "onef": np.ones((DK,), dtype=f),
        "ebias": np.full((1,), np.log(32.0), dtype=f),
    }


def kernel(
    queries, keys, values, wq, bq, wk, bk, wv, bv, wo, bo, has_mask, **_unused
) -> np.ndarray:
    has_mask = bool(np.asarray(has_mask).item())
    nc = _get_nc(has_mask)
    in_maps = [
        _core_inputs(queries, keys, values, wq, bq, wk, bk, wv, bv, wo, c)
        for c in range(8)
    ]
    res = run_bass_kernel_spmd(nc, in_maps, core_ids=list(range(8)))
    global LAST_RESULT
    LAST_RESULT = res
    parts = [np.asarray(res.results[c]["y"], dtype=np.float32) for c in range(8)]
    out = np.stack(
        [parts[2 * b] + parts[2 * b + 1] + np.asarray(bo, dtype=np.float32) for b in range(4)]
    )
    return out.astype(np.float32)


# revision 44
# speedup vs baseline: 1.0007x; 1.0007x over previous
"""Multi-head causal attention on 8 Trainium2 NeuronCores.

Sharding: core c handles batch b = c//2 and head-group g = c%2 (4 heads).
Each core computes a partial output  y_part = sum_{h in group} attn_h @ wo_h
for its batch; the host sums the two head-group partials per batch and adds bo.
Pure data-SPMD: one NEFF, per-core input slices, no collectives.

Per-core kernel layout choices:
  - Inputs are host-transposed to x^T [D=512, S=2048] so the projection
    contraction dim (D) lands on SBUF partitions with contiguous DMA.
  - Q/K projections are head-pair-packed: out psum [128 = 2x64 dims, q].
  - Scores are computed transposed (S^T[k, q] blocks) so the exp output P^T
    is directly the moving operand of the P@V matmul - no on-chip transposes.
  - P^T is stored fp8e4m3 (exp writes it directly); the P@V matmul runs in
    DoubleRow perf mode: lhsT = (V_hi, V_lo) fp8 pair (hi/lo split of bf16 V,
    computed on GPSIMD), rhs = P^T duplicated via a stride-0 AP. One DR
    matmul per (j, head) does the full-precision-V P@V at half PE cost.
    A 16th fp8 column of 1.0 in V_hi emits the softmax denominator row.
  - Causal: block-skipping at 128-row granularity, q-range trimming per
    diagonal block. The diagonal 128x128 is masked by a uint32 bitwise-AND
    on DVE (zeroes fp8 bytes above the diagonal). The attn accumulation
    group grows in width as j decreases (has_written overwrite semantics),
    so no memsets are needed for the trimmed regions.
  - y is written bf16, one batched DMA per 512-row slab.
"""

import os
import sys

import ml_dtypes
import numpy as np

try:
    import concourse.bass as bass
except ImportError:  # pragma: no cover
    sys.path.insert(0, "/opt/trn_rl_repo")
    import concourse.bass as bass

import concourse.bacc as bacc
import concourse.mybir as mybir
import concourse.tile as tile
from concourse.bass_utils import run_bass_kernel_spmd

F32 = mybir.dt.float32
F32R = mybir.dt.float32r
BF16 = mybir.dt.bfloat16
F8 = mybir.dt.float8e4
U32 = mybir.dt.uint32

S = 2048          # sequence length
D = 512           # model dim
DK = 64           # head dim (= dv)
NH = 4            # heads per core
NPAIR = 2         # head pairs per core
P = 128           # partitions
NQB = S // 512    # 512-wide q column blocks
NKB = S // P      # 128-wide k row blocks
NSB = S // P      # 128-wide s row blocks
NCHUNK = D // P   # 128-row chunks of the model dim
VW = 80           # padded fp8 V' row width (65 used; stride % 16 == 0)


def build_nc(has_mask: bool, cfg: dict | None = None) -> bass.Bass:
    cfg = {**{"sc_bufs": 2, "at_bufs": 1, "pt_bufs": 14, "nrm_bufs": 4, "shift_dma": "sync", "prefetch": 8}, **(cfg or {})}
    nc = bacc.Bacc("TRN2", target_bir_lowering=False)

    qT = nc.dram_tensor("qt", [D, S], BF16, kind="ExternalInput")
    kT = nc.dram_tensor("kt", [D, S], BF16, kind="ExternalInput")
    vT = nc.dram_tensor("vt", [D, S], BF16, kind="ExternalInput")
    wq = nc.dram_tensor("wq", [D, NH * DK], BF16, kind="ExternalInput")
    wk = nc.dram_tensor("wk", [D, NH * DK], BF16, kind="ExternalInput")
    wv = nc.dram_tensor("wv", [D, NH * DK], BF16, kind="ExternalInput")
    bq = nc.dram_tensor("bq", [P, NPAIR], F32, kind="ExternalInput")
    bk = nc.dram_tensor("bk", [P, NPAIR], F32, kind="ExternalInput")
    bv = nc.dram_tensor("bv", [NH * DK], F32, kind="ExternalInput")
    wo = nc.dram_tensor("wo", [NPAIR, P, D], BF16, kind="ExternalInput")
    tri = nc.dram_tensor("tri", [P, P // 4], U32, kind="ExternalInput")
    onef = nc.dram_tensor("onef", [DK], F32R, kind="ExternalInput")
    ebias = nc.dram_tensor("ebias", [1], F32, kind="ExternalInput")
    y = nc.dram_tensor("y", [S, D], BF16, kind="ExternalOutput")

    y4 = y.rearrange("(i b p) d -> i p b d", p=P, b=4)

    with tile.TileContext(nc) as tc, nc.allow_low_precision(reason="fp8 P/V attention path; bf16 outputs"):
        with tc.tile_pool(name="persist", bufs=1) as persist:
            QT = persist.tile([P, NPAIR, S], F8, tag="QT")
            KT = persist.tile([P, NPAIR, S], BF16, tag="KT")
            # fp8 K hi/lo pairs: [dims, pair, j, {hi,lo}, 128]
            KT8 = persist.tile([P, NPAIR, NKB, 2, P], F8, tag="KT8")
            Vp = persist.tile([P, NKB, NH, DK], F32, tag="Vp")
            # fp8 V' hi/lo pairs: [k, j, {hi,lo}, head, 80] (cols 0:64 = V, col 64 = 1/0)
            Vp8 = persist.tile([P, NKB, 2, NH, VW], F8, tag="Vp8")
            aT0 = persist.tile([P, S], BF16, tag="aT0")
            aT1 = persist.tile([P, S], BF16, tag="aT1")
            attnT = [aT0, aT1]
            wo_sb = persist.tile([P, NPAIR, D], BF16, tag="wo")
            bq_sb = persist.tile([P, NPAIR], F32, tag="bq")
            bk_sb = persist.tile([P, NPAIR], F32, tag="bk")
            bv_sb = persist.tile([P, NH, DK], F32, tag="bv")
            tri_sb = persist.tile([P, P // 4], U32, tag="tri")
            onef_sb = persist.tile([DK + 1, DK], F32R, tag="onef")
            eb_sb = persist.tile([P, 1], F32, tag="eb")

            with (
                tc.tile_pool(name="inp", bufs=1) as inp,
                tc.tile_pool(name="wts", bufs=1) as wts,
                tc.tile_pool(name="scps", bufs=cfg["sc_bufs"], space="PSUM") as scps,
                tc.tile_pool(name="atps", bufs=cfg["at_bufs"], space="PSUM") as atps,
                tc.tile_pool(name="prepps", bufs=2, space="PSUM") as prepps,
                tc.tile_pool(name="ptp", bufs=cfg["pt_bufs"]) as ptp,
                tc.tile_pool(name="nrm", bufs=cfg["nrm_bufs"]) as nrm,
                tc.tile_pool(name="outp", bufs=2) as outp,
            ):
                qT_sb = inp.tile([P, NCHUNK, S], BF16, tag="qT")
                kT_sb = inp.tile([P, NCHUNK, S], BF16, tag="kT")
                vT_sb = inp.tile([P, NCHUNK, S], BF16, tag="vT")
                wq_sb = wts.tile([P, NCHUNK, NH * DK], BF16, tag="wq")
                wk_sb = wts.tile([P, NCHUNK, NH * DK], BF16, tag="wk")
                wv_sb = wts.tile([P, NCHUNK, NH * DK], BF16, tag="wv")
                # One combined DMA per (tensor, 512-col block), emitted in
                # criticality order: HWDGE charges a serial per-DMA trigger
                # cost, so few big transfers with the slab-0 data first.
                kT4 = kT.rearrange("(c p) s -> p c s", p=P)
                qT4 = qT.rearrange("(c p) s -> p c s", p=P)
                vT4 = vT.rearrange("(c p) s -> p c s", p=P)

                def in_dma(dst, src, qb):
                    cols = slice(512 * qb, 512 * (qb + 1))
                    nc.sync.dma_start(out=dst[..., cols], in_=src[..., cols])

                # warm the PE pstate ramp during the input-DMA wait: ~4us of
                # garbage matmuls on a memset tile into a scratch psum bank
                wu = ptp.tile([P, 512], BF16, tag="wu", name="wu")
                nc.gpsimd.memset(wu, 0.0)
                wups = prepps.tile([P, 512], F32, tag="pp", name="wups")
                for _ in range(12):
                    nc.tensor.matmul(wups, lhsT=wu[:, 0:P], rhs=wu, start=True, stop=True)
                # preload the Exp activation table off the critical path
                wua = ptp.tile([1, 1], BF16, tag="wua", name="wua")
                with tc.high_priority(offset=(1 << 20) + 1000):
                    nc.scalar.activation(out=wua, in_=wu[0:1, 0:1], func=mybir.ActivationFunctionType.Exp)

                nc.sync.dma_start(out=wk_sb, in_=wk.rearrange("(c p) m -> p c m", p=P))
                in_dma(kT_sb, kT4, 0)
                nc.sync.dma_start(out=wq_sb, in_=wq.rearrange("(c p) m -> p c m", p=P))
                in_dma(qT_sb, qT4, 0)
                nc.sync.dma_start(out=bq_sb, in_=bq[:, :])
                nc.sync.dma_start(out=bk_sb, in_=bk[:, :])
                nc.sync.dma_start(out=wv_sb, in_=wv.rearrange("(c p) m -> p c m", p=P))
                in_dma(vT_sb, vT4, 0)
                nc.sync.dma_start(out=bv_sb, in_=bv.rearrange("(h c) -> h c", h=NH).partition_broadcast(P))
                nc.sync.dma_start(out=tri_sb, in_=tri[:, :])
                nc.sync.dma_start(out=onef_sb[DK : DK + 1, :], in_=onef[:].partition_broadcast(1))
                nc.sync.dma_start(out=eb_sb, in_=ebias[:].partition_broadcast(P))
                # V' ones/zeros columns via cheap GPSIMD memsets (a strided
                # 1-byte DMA here would serialize the startup DMA queue)
                nc.gpsimd.memset(Vp8[:, :, 0, :, DK], 1.0)
                nc.gpsimd.memset(Vp8[:, :, 1, :, DK], 0.0)
                for qb in (1, 2):
                    in_dma(kT_sb, kT4, qb)
                    in_dma(qT_sb, qT4, qb)
                    in_dma(vT_sb, vT4, qb)
                nc.sync.dma_start(out=wo_sb, in_=wo.rearrange("e p m -> p e m"))
                for qb in (3,):
                    in_dma(kT_sb, kT4, qb)
                    in_dma(qT_sb, qT4, qb)
                    in_dma(vT_sb, vT4, qb)

                # pair-packed projection of one 512-col block: psum [128, 512]
                def proj_block(x_sb, w_sb, b_sb, out_sb, pair, qb):
                    ps = prepps.tile([P, 512], F32, tag="pp", name="pj")
                    for c in range(NCHUNK):
                        nc.tensor.matmul(
                            ps,
                            lhsT=(w_sb[:, c, P * pair : P * (pair + 1)]),
                            rhs=(x_sb[:, c, 512 * qb : 512 * (qb + 1)]),
                            start=(c == 0),
                            stop=(c == NCHUNK - 1),
                        )
                    # the bias-evac feeds the scores pipeline: rank it with
                    # the pipeline so DVE doesn't sit on it behind other work
                    with tc.high_priority(offset=1 << 20):
                        nc.vector.tensor_scalar_add(
                            out=out_sb[:, pair, 512 * qb : 512 * (qb + 1)],
                            in0=ps,
                            scalar1=b_sb[:, pair : pair + 1],
                        )

                # fp8 hi/lo split of K block (pair, qb) on GPSIMD; also a
                # scores-pipeline producer, so pipeline priority
                def ksplit_block(pair, qb):
                    kb = KT[:, pair, 512 * qb : 512 * (qb + 1)].rearrange(
                        "p (j c) -> p j c", c=P
                    )
                    with tc.high_priority(offset=1 << 20):
                        nc.gpsimd.tensor_copy(
                            out=KT8[:, pair, 4 * qb : 4 * qb + 4, 0, :], in_=kb
                        )
                        nc.gpsimd.tensor_tensor(
                            out=KT8[:, pair, 4 * qb : 4 * qb + 4, 1, :],
                            in0=kb,
                            in1=KT8[:, pair, 4 * qb : 4 * qb + 4, 0, :],
                            op=mybir.AluOpType.subtract,
                        )

                # V natural: per s-block psum [128 s, 256 = 4 heads x 64]
                def vproj_block(sb):
                    ps_full = prepps.tile([P, 512], F32, tag="pp", name="pjv")
                    ps = ps_full[:, 0 : NH * DK]
                    for c in range(NCHUNK):
                        nc.tensor.matmul(
                            ps,
                            lhsT=(vT_sb[:, c, P * sb : P * (sb + 1)]),
                            rhs=(wv_sb[:, c]),
                            start=(c == 0),
                            stop=(c == NCHUNK - 1),
                        )
                    nc.vector.tensor_tensor(
                        out=Vp[:, sb],
                        in0=ps.rearrange("p (h c) -> p h c", h=NH),
                        in1=bv_sb,
                        op=mybir.AluOpType.add,
                    )

                # fp8 hi/lo split of V' block sb (GPSIMD, SBUF-only)
                def vsplit_block(sb):
                    nc.gpsimd.tensor_copy(
                        out=Vp8[:, sb, 0, :, 0:DK], in_=Vp[:, sb]
                    )
                    nc.gpsimd.tensor_tensor(
                        out=Vp8[:, sb, 1, :, 0:DK],
                        in0=Vp[:, sb],
                        in1=Vp8[:, sb, 0, :, 0:DK],
                        op=mybir.AluOpType.subtract,
                    )

                # output projection for one 128-row s-block into the slab tile
                def oproj_block(sb, y_sb):
                    yp = prepps.tile([P, 512], F32, tag="pp", name="yp")
                    for pair in range(NPAIR):
                        nc.tensor.matmul(
                            yp,
                            lhsT=(attnT[pair][:, P * sb : P * (sb + 1)]),
                            rhs=(wo_sb[:, pair]),
                            start=(pair == 0),
                            stop=(pair == NPAIR - 1),
                        )
                    nc.vector.tensor_copy(out=y_sb[:, sb % 4, :], in_=yp)

                # ---- attention slabs with dripped prep work ----
                # work items are (due, weight, fn): fn must be EMITTED before
                # slab `due` starts (Tile derives deps from emission order).
                # The pacer spreads items evenly over the global j-slot
                # budget, weighted by each item's PE cost, so per-j PE load
                # stays just under the ACT exp cadence.
                work = []
                total_jslots = 2 * sum(4 * i + 4 for i in range(NQB)) if has_mask else 2 * NQB * NKB
                pace = {"jdone": 0, "popped_w": 0.0, "queued_w": 0.0}

                def drip_due(I):
                    while work and work[0][0] <= I:
                        pace["popped_w"] += work[0][1]
                        work.pop(0)[2]()

                def drip_pace():
                    pace["jdone"] += 1
                    target = pace["queued_w"] * pace["jdone"] / total_jslots
                    while work and pace["popped_w"] < target:
                        pace["popped_w"] += work[0][1]
                        work.pop(0)[2]()

                def queue(due, weight, fn):
                    work.append((due, weight, fn))
                    pace["queued_w"] += weight

                def scores_exp(pair, I, j):
                    t = j - 4 * I
                    qlo = 128 * t if (has_mask and t > 0) else 0
                    sc = scps.tile([P, 2, 512], F32, tag="sc", name="sc")
                    pt = ptp.tile([P, 2, 512], F8, tag="pt", name="pt")
                    for hh in range(2):
                        qv = QT[64 * hh : 64 * hh + 64, pair, 512 * I + qlo : 512 * (I + 1)]
                        nc.tensor.matmul(
                            sc[:, hh, qlo:],
                            lhsT=KT8[64 * hh : 64 * hh + 64, pair, j, :, :],
                            rhs=qv.unsqueeze(1).broadcast_to([64, 2, 512 - qlo]),
                            start=True,
                            stop=True,
                            tile_position=(64 * hh, 0),
                            perf_mode=mybir.MatmulPerfMode.DoubleRow,
                        )
                    # one ACT op covers both heads; scores are x32 (fp8 Q was
                    # prescaled), compensated by the exp scale; the +ln32 bias
                    # recenters P in the fp8 range and cancels in the softmax
                    nc.scalar.activation(
                        out=pt[:, :, qlo:],
                        in_=sc[:, :, qlo:],
                        func=mybir.ActivationFunctionType.Exp,
                        bias=eb_sb[:, 0:1],
                        scale=0.03125,
                    )
                    if has_mask and 0 <= t:
                        for hh in range(2):
                            nc.vector.tensor_tensor(
                                out=pt[:, hh, qlo : qlo + P].bitcast(U32),
                                in0=pt[:, hh, qlo : qlo + P].bitcast(U32),
                                in1=tri_sb,
                                op=mybir.AluOpType.bitwise_and,
                            )
                    return (pt, qlo)

                def attn_mm(at, pair, I, j, first, last, ptq):
                    # descending j normally (start=True on the narrow
                    # diagonal block, widths grow via has_written overwrite);
                    # the last slab runs ascending (start=True on full-width
                    # j=0) so its final exp is the small diagonal block and
                    # the tail normalize chain starts earlier
                    pt, qlo = ptq
                    for hh in range(2):
                        rhs = pt[:, hh, qlo:].unsqueeze(1).broadcast_to([P, 2, 512 - qlo])
                        nc.tensor.matmul(
                            at[0 : DK + 1, hh, qlo:],
                            lhsT=Vp8[:, j, :, 2 * pair + hh, 0 : DK + 1],
                            rhs=rhs,
                            start=first,
                            stop=last,
                            perf_mode=mybir.MatmulPerfMode.DoubleRow,
                        )

                def normalize(at, pair, I):
                    # reciprocal straight off the psum denominator row while
                    # the numerators evacuate; the broadcast ones-matmul then
                    # REUSES the at psum banks (already drained), so the
                    # scores-pool rotation never blocks on this chain
                    rec = nrm.tile([DK + 1, 2, 512], F32R, tag="rec")
                    nc.vector.reciprocal(out=rec[DK : DK + 1], in_=at[DK : DK + 1])
                    anum = nrm.tile([DK, 2, 512], F32, tag="anum")
                    if pair == 1 and I == NQB - 1:
                        # tail: ACT is idle; shortens the last at-release chain
                        nc.scalar.copy(out=anum, in_=at[0:DK])
                    else:
                        nc.vector.tensor_copy(out=anum, in_=at[0:DK])
                    for hh in range(2):
                        nc.tensor.matmul(
                            at[0:DK, hh, :],
                            lhsT=onef_sb[DK : DK + 1, :],
                            rhs=rec[DK : DK + 1, hh, :],
                            start=True,
                            stop=True,
                            tile_position=(DK, 0),
                        )
                    nc.vector.tensor_tensor(
                        out=attnT[pair][0:DK, 512 * I : 512 * (I + 1)],
                        in0=anum[0:DK, 0],
                        in1=at[0:DK, 0],
                        op=mybir.AluOpType.mult,
                    )
                    tmp = nrm.tile([DK, 512], BF16, tag="tmp")
                    if I == NQB - 1:
                        # tail: per-chunk multiply+shift so the final output
                        # projections start as soon as each chunk lands
                        for cc in range(4):
                            cs = slice(128 * cc, 128 * (cc + 1))
                            nc.vector.tensor_tensor(
                                out=tmp[:, cs],
                                in0=anum[0:DK, 1, cs],
                                in1=at[0:DK, 1, cs],
                                op=mybir.AluOpType.mult,
                            )
                            getattr(nc, cfg["shift_dma"]).dma_start(
                                out=attnT[pair][DK:P, 512 * I + 128 * cc : 512 * I + 128 * (cc + 1)],
                                in_=tmp[:, cs],
                            )
                    else:
                        nc.vector.tensor_tensor(
                            out=tmp,
                            in0=anum[0:DK, 1],
                            in1=at[0:DK, 1],
                            op=mybir.AluOpType.mult,
                        )
                        getattr(nc, cfg["shift_dma"]).dma_start(
                            out=attnT[pair][DK:P, 512 * I : 512 * (I + 1)], in_=tmp
                        )

                carry = {}

                def slab(pair, I):
                    jmax = 4 * I + 3 if has_mask else NKB - 1
                    if I == NQB - 1:
                        js = list(range(jmax + 1))
                    else:
                        js = list(range(jmax, -1, -1))

                    def get_pt(j):
                        key = (pair, I, j)
                        if key in carry:
                            return carry.pop(key)
                        return scores_exp(pair, I, j)

                    at = atps.tile([P, 2, 512], F32, tag="at", name="at")
                    for idx, j in enumerate(js):
                        with tc.high_priority(offset=1 << 20):
                            pt = get_pt(j)
                            attn_mm(at, pair, I, j, idx == 0, idx == len(js) - 1, pt)
                    with tc.high_priority(offset=1 << 20):
                        normalize(at, pair, I)

                # prologue: pair-0 K/Q for slab 0 first (exp critical
                # path), then V prep, then ALL remaining projections in slab
                # order. Everything here is low priority: the scheduler
                # backfills PE with it whenever the high-priority attention
                # pipeline has nothing ready.
                k_qbs0 = [0] if has_mask else list(range(NQB))
                v_sbs0 = list(range(4)) if has_mask else list(range(NSB))
                def ksplit_chunked(pair, qb):
                    for j in range(4 * qb + 3, 4 * qb - 1, -1):
                        kb = KT[:, pair, P * j : P * (j + 1)]
                        with tc.high_priority(offset=1 << 20):
                            nc.gpsimd.tensor_copy(out=KT8[:, pair, j, 0, :], in_=kb)
                            nc.gpsimd.tensor_tensor(
                                out=KT8[:, pair, j, 1, :],
                                in0=kb,
                                in1=KT8[:, pair, j, 0, :],
                                op=mybir.AluOpType.subtract,
                            )

                for qb in k_qbs0:
                    proj_block(kT_sb, wk_sb, bk_sb, KT, 0, qb)
                    ksplit_chunked(0, qb)
                proj_block(qT_sb, wq_sb, bq_sb, QT, 0, 0)
                for sb in reversed(v_sbs0):
                    # descending: the slab-0 attn loop consumes j=jmax first
                    vproj_block(sb)
                    vsplit_block(sb)
                for qb in k_qbs0:
                    proj_block(kT_sb, wk_sb, bk_sb, KT, 1, qb)
                    ksplit_block(1, qb)
                proj_block(qT_sb, wq_sb, bq_sb, QT, 1, 0)
                for I in range(1, NQB):
                    if has_mask:
                        proj_block(kT_sb, wk_sb, bk_sb, KT, 0, I)
                        ksplit_block(0, I)
                        proj_block(qT_sb, wq_sb, bq_sb, QT, 0, I)
                        proj_block(kT_sb, wk_sb, bk_sb, KT, 1, I)
                        ksplit_block(1, I)
                        for sb in range(4 * I, 4 * I + 4):
                            vproj_block(sb)
                            vsplit_block(sb)
                        proj_block(qT_sb, wq_sb, bq_sb, QT, 1, I)
                    else:
                        proj_block(qT_sb, wq_sb, bq_sb, QT, 0, I)
                        proj_block(qT_sb, wq_sb, bq_sb, QT, 1, I)

                for I in range(NQB):
                    # oproj/DMA for slab I-1 were queued after its normalize
                    # was emitted (Tile derives deps from emission order)
                    drip_due(I)
                    slab(0, I)
                    slab(1, I)
                    if I > 0:
                        ys = outp.tile([P, 4, 512], BF16, tag="y", name="ys")
                        for sb in range(4 * (I - 1), 4 * I):
                            queue(I + 1, 430, lambda sb=sb, ys=ys: oproj_block(sb, ys))
                        queue(I + 1, 0, lambda I=I, ys=ys: nc.sync.dma_start(out=y4[I - 1], in_=ys))
                    if I + 1 < NQB:
                        # prefetch the next slab's first scores/exp blocks so
                        # ACT keeps streaming across the slab boundary
                        jm = 4 * (I + 1) + 3 if has_mask else NKB - 1
                        npf = cfg.get("prefetch", 8)
                        if I + 1 == NQB - 1:
                            pjs = list(range(0, min(npf, jm + 1)))
                        else:
                            pjs = list(range(jm, max(jm - npf, -1), -1))
                        for j in pjs:
                            with tc.high_priority(offset=1 << 20):
                                carry[(0, I + 1, j)] = scores_exp(0, I + 1, j)
                # drain any remaining prep, then the final slab's output
                # proj: ACT evacuation (idle at the tail) + per-block DMAs.
                # Junk matmuls keep the PE pstate warm through the last
                # normalize chain so the final oprojs run at full rate.
                for _ in range(10):
                    nc.tensor.matmul(wups, lhsT=wu[:, 0:P], rhs=wu, start=True, stop=True)
                drip_due(NQB + 1)
                ys = outp.tile([P, 4, 512], BF16, tag="y", name="ys")
                for sb in range(4 * (NQB - 1), 4 * NQB):
                    yp = prepps.tile([P, 512], F32, tag="pp", name="yp")
                    for pair in range(NPAIR):
                        nc.tensor.matmul(
                            yp,
                            lhsT=(attnT[pair][:, P * sb : P * (sb + 1)]),
                            rhs=(wo_sb[:, pair]),
                            start=(pair == 0),
                            stop=(pair == NPAIR - 1),
                        )
                    nc.scalar.copy(out=ys[:, sb % 4, :], in_=yp)
                    nc.sync.dma_start(out=y4[NQB - 1][:, sb % 4], in_=ys[:, sb % 4, :])
    nc.compile()
    return nc


_NC_CACHE: dict = {}
LAST_RESULT = None


def _get_nc(has_mask: bool) -> bass.Bass:
    key = bool(has_mask)
    if key not in _NC_CACHE:
        _NC_CACHE[key] = build_nc(key)
    return _NC_CACHE[key]


def _core_inputs(queries, keys, values, wq, bq, wk, bk, wv, bv, wo, core: int) -> dict:
    b, g = core // 2, core % 2
    heads = list(range(4 * g, 4 * g + 4))
    f = np.float32
    bf = ml_dtypes.bfloat16
    f8 = ml_dtypes.float8_e4m3
    scale = np.float32(1.0 / np.sqrt(DK))

    def packw(w, s=1.0):
        return np.ascontiguousarray(
            np.concatenate([w[h] for h in heads], axis=1) * np.float32(s)
        ).astype(bf)

    def packb(bvec, s=1.0):
        cols = [np.concatenate([bvec[heads[2 * p]], bvec[heads[2 * p + 1]]]) for p in range(NPAIR)]
        return np.ascontiguousarray(np.stack(cols, axis=1) * np.float32(s), dtype=f)

    return {
        "qt": np.ascontiguousarray(queries[b].T).astype(bf),
        "kt": np.ascontiguousarray(keys[b].T).astype(bf),
        "vt": np.ascontiguousarray(values[b].T).astype(bf),
        "wq": packw(wq, scale * 32.0),
        "wk": packw(wk),
        "wv": packw(wv),
        "bq": packb(bq, scale * 32.0),
        "bk": packb(bk),
        "bv": np.ascontiguousarray(np.concatenate([bv[h] for h in heads]), dtype=f),
        "wo": np.ascontiguousarray(
            np.stack([wo[DK * heads[2 * p] : DK * (heads[2 * p] + 2)] for p in range(NPAIR)])
        ).astype(bf),
        "tri": np.ascontiguousarray(
            np.triu(np.full((P, P), 0xFF, dtype=np.uint8))
        ).view(np.uint32),
        "onef": np.ones((DK,), dtype=f),
        "ebias": np.full((1,), np.log(32.0), dtype=f),
    }


def kernel(
    queries, keys, values, wq, bq, wk, bk, wv, bv, wo, bo, has_mask, **_unused
) -> np.ndarray:
    has_mask = bool(np.asarray(has_mask).item())
    nc = _get_nc(has_mask)
    in_maps = [
        _core_inputs(queries, keys, values, wq, bq, wk, bk, wv, bv, wo, c)
        for c in range(8)
    ]
    res = run_bass_kernel_spmd(nc, in_maps, core_ids=list(range(8)))
    global LAST_RESULT
    LAST_RESULT = res
    parts = [np.asarray(res.results[c]["y"], dtype=np.float32) for c in range(8)]
    out = np.stack(
        [parts[2 * b] + parts[2 * b + 1] + np.asarray(bo, dtype=np.float32) for b in range(4)]
    )
    return out.astype(np.float32)
